# revision 8
# baseline (speedup 1.0000x reference)
"""Trainium2 Bass kernel for nn_AttentionAggregator3 (GNN message passing).

Takes FULL unsharded inputs, shards target-node rows (N=2048) across 8
NeuronCores (256 rows each), replicates neibs + weights, runs an
all-transposed dataflow per core, gathers the full (2048, 256) output.

Note: the reference computes an edge-embedding MLP whose result is never
used (dead code), so edge_emb is accepted and ignored.
"""

import sys

sys.path.insert(0, "/opt/trn_rl_repo")

import numpy as np

import concourse.bass as bass
import concourse.tile as tile
from concourse import bacc, mybir
from concourse.bass_utils import run_bass_kernel_spmd

N = 2048        # target nodes
NN = 2048       # neighbor pool
D = 256         # d_in
H = 32          # hidden
DO = 256        # d_out
NCORES = 8
R = N // NCORES  # rows per core = 256

F32 = mybir.dt.float32
F32R = mybir.dt.float32r

def build_kernel():
    nc = bacc.Bacc("TRN2")

    # ---- DRAM I/O (per-core shapes; pre-tiled on host into [128, c, n]) ----
    xt = nc.dram_tensor("xt", [128, 2, R], F32R, kind="ExternalInput")          # x_shard.T
    neibst = nc.dram_tensor("neibst", [128, 2, NN], F32R, kind="ExternalInput")  # neibs.T
    neibs = nc.dram_tensor("neibs", [128, 16, D], F32R, kind="ExternalInput")    # neibs
    maskt = nc.dram_tensor("maskt", [128, 16, R], F32, kind="ExternalInput")    # mask_shard.T
    axw1 = nc.dram_tensor("axw1", [128, 2, H], F32R, kind="ExternalInput")
    axb1 = nc.dram_tensor("axb1", [H, 1], F32, kind="ExternalInput")
    axw2 = nc.dram_tensor("axw2", [H, H], F32R, kind="ExternalInput")
    axb2 = nc.dram_tensor("axb2", [H, 1], F32, kind="ExternalInput")
    anw1 = nc.dram_tensor("anw1", [128, 2, H], F32R, kind="ExternalInput")
    anb1 = nc.dram_tensor("anb1", [H, 1], F32, kind="ExternalInput")
    anw2 = nc.dram_tensor("anw2", [H, H], F32R, kind="ExternalInput")
    anb2 = nc.dram_tensor("anb2", [H, 1], F32, kind="ExternalInput")
    fcw = nc.dram_tensor("fcw", [128, 4, DO], F32R, kind="ExternalInput")        # fcx_w
    fcb = nc.dram_tensor("fcb", [128, 2, 1], F32, kind="ExternalInput")         # fcx_b
    onescol = nc.dram_tensor("onescol", [128, 1], F32R, kind="ExternalInput")
    onesrow = nc.dram_tensor("onesrow", [1, 128], F32R, kind="ExternalInput")
    out = nc.dram_tensor("out", [128, 2, R], F32, kind="ExternalOutput")        # out_shard.T

    AF = mybir.ActivationFunctionType

    with tile.TileContext(nc) as tc:
        with (
            tc.tile_pool(name="big", bufs=1) as big,
            tc.tile_pool(name="small", bufs=1) as small,
            tc.tile_pool(name="work", bufs=4) as work,
            tc.tile_pool(name="psA", bufs=2, space="PSUM") as psA,
            tc.tile_pool(name="psB", bufs=2, space="PSUM") as psB,
            tc.tile_pool(name="psC", bufs=1, space="PSUM") as psC,
        ):
            # ---- load everything into SBUF ----
            xt_sb = big.tile([128, 2, R], F32R, tag="xt")
            nc.sync.dma_start(out=xt_sb, in_=xt[:])
            neibst_sb = big.tile([128, 2, NN], F32R, tag="neibst")
            nc.sync.dma_start(out=neibst_sb, in_=neibst[:])
            neibs_sb = big.tile([128, 16, D], F32R, tag="neibs")
            nc.sync.dma_start(out=neibs_sb, in_=neibs[:])
            maskt_sb = big.tile([128, 16, R], F32, tag="maskt")
            nc.sync.dma_start(out=maskt_sb, in_=maskt[:])
            fcw_sb = big.tile([128, 4, DO], F32R, tag="fcw")
            nc.sync.dma_start(out=fcw_sb, in_=fcw[:])

            axw1_sb = small.tile([128, 2, H], F32R, tag="axw1")
            nc.sync.dma_start(out=axw1_sb, in_=axw1[:])
            anw1_sb = small.tile([128, 2, H], F32R, tag="anw1")
            nc.sync.dma_start(out=anw1_sb, in_=anw1[:])
            axw2_sb = small.tile([H, H], F32R, tag="axw2")
            nc.sync.dma_start(out=axw2_sb, in_=axw2[:])
            anw2_sb = small.tile([H, H], F32R, tag="anw2")
            nc.sync.dma_start(out=anw2_sb, in_=anw2[:])
            axb1_sb = small.tile([H, 1], F32, tag="axb1")
            nc.sync.dma_start(out=axb1_sb, in_=axb1[:])
            axb2_sb = small.tile([H, 1], F32, tag="axb2")
            nc.sync.dma_start(out=axb2_sb, in_=axb2[:])
            anb1_sb = small.tile([H, 1], F32, tag="anb1")
            nc.sync.dma_start(out=anb1_sb, in_=anb1[:])
            anb2_sb = small.tile([H, 1], F32, tag="anb2")
            nc.sync.dma_start(out=anb2_sb, in_=anb2[:])
            fcb_sb = small.tile([128, 2, 1], F32, tag="fcb")
            nc.sync.dma_start(out=fcb_sb, in_=fcb[:])

            ones_col = small.tile([128, 1], F32R, tag="ones_col")   # lhsT for col-sum
            nc.sync.dma_start(out=ones_col, in_=onescol[:])
            ones_row = small.tile([1, 128], F32R, tag="ones_row")    # lhsT for bcast
            nc.sync.dma_start(out=ones_row, in_=onesrow[:])

            # ---- x_attT = (tanh(xT.T @ w1 + b1) @ w2 + b2).T   [H, R] ----
            h1x_ps = psA.tile([H, 512], F32, tag="mlp", name="h1x_ps")[:, :R]
            for k in range(2):
                nc.tensor.matmul(
                    out=h1x_ps,
                    lhsT=axw1_sb[:, k, :],
                    rhs=xt_sb[:, k, :],
                    start=(k == 0),
                    stop=(k == 1),
                )
            h1x_sb = small.tile([H, R], F32R, tag="h1x")
            nc.scalar.activation(out=h1x_sb, in_=h1x_ps, func=AF.Tanh, bias=axb1_sb)
            xatt_ps = psA.tile([H, 512], F32, tag="mlp", name="xatt_ps")[:, :R]
            nc.tensor.matmul(
                out=xatt_ps, lhsT=axw2_sb[:], rhs=h1x_sb,
                start=True, stop=True,
            )
            xatt_sb = small.tile([H, R], F32R, tag="xatt")
            nc.scalar.activation(out=xatt_sb, in_=xatt_ps, func=AF.Identity, bias=axb2_sb)

            # ---- neib_attT  [H, NN] ----
            h1n_sb = big.tile([H, NN], F32R, tag="h1n")
            for j in range(NN // 512):
                h1n_ps = psA.tile([H, 512], F32, tag="mlp")
                for k in range(2):
                    nc.tensor.matmul(
                        out=h1n_ps,
                        lhsT=anw1_sb[:, k, :],
                        rhs=neibst_sb[:, k, j * 512:(j + 1) * 512],
                        start=(k == 0),
                        stop=(k == 1),
                    )
                nc.scalar.activation(
                    out=h1n_sb[:, j * 512:(j + 1) * 512], in_=h1n_ps,
                    func=AF.Tanh, bias=anb1_sb,
                )
            natt_sb = big.tile([H, NN], F32R, tag="natt")
            for j in range(NN // 512):
                natt_ps = psA.tile([H, 512], F32, tag="mlp")
                nc.tensor.matmul(
                    out=natt_ps, lhsT=anw2_sb[:],
                    rhs=h1n_sb[:, j * 512:(j + 1) * 512],
                    start=True, stop=True,
                )
                nc.scalar.activation(
                    out=natt_sb[:, j * 512:(j + 1) * 512], in_=natt_ps,
                    func=AF.Identity, bias=anb2_sb,
                )

            # ---- scores -> e = exp(scores*mask), sums, aggT accumulation ----
            e_all = big.tile([128, 16, R], F32R, tag="e_all")
            sum_ps = psC.tile([1, R], F32, tag="sum_ps")
            agg_ps0 = psC.tile([128, R], F32, tag="agg_ps0")
            agg_ps1 = psC.tile([128, R], F32, tag="agg_ps1")
            agg_pss = [agg_ps0, agg_ps1]

            for m in range(16):
                sc_ps = psB.tile([128, R], F32, tag="scfc")
                nc.tensor.matmul(
                    out=sc_ps,
                    lhsT=natt_sb[:, m * 128:(m + 1) * 128],
                    rhs=xatt_sb[:],
                    start=True, stop=True,
                )
                st_sb = work.tile([128, R], F32, tag="st")
                nc.vector.tensor_mul(st_sb, sc_ps, maskt_sb[:, m, :])
                nc.scalar.activation(out=e_all[:, m, :], in_=st_sb, func=AF.Exp)
                # denominator: accumulate column-sums of e (sum over neighbors)
                nc.tensor.matmul(
                    out=sum_ps, lhsT=ones_col[:], rhs=e_all[:, m, :],
                    start=(m == 0), stop=(m == 15),
                )
                # aggT += neibs_chunk.T-style accumulation: [feat, rows]
                for i in range(2):
                    nc.tensor.matmul(
                        out=agg_pss[i],
                        lhsT=neibs_sb[:, m, i * 128:(i + 1) * 128],
                        rhs=e_all[:, m, :],
                        start=(m == 0), stop=(m == 15),
                    )

            # ---- normalize + sigmoid ----
            recip_sb = small.tile([1, R], F32R, tag="recip")
            with nc.allow_low_precision(reason="recip feeds f32r matmul"):
                nc.vector.reciprocal(out=recip_sb, in_=sum_ps)
            rb_ps = psB.tile([128, R], F32, tag="scfc")
            nc.tensor.matmul(
                out=rb_ps, lhsT=ones_row[:], rhs=recip_sb[:],
                start=True, stop=True,
            )
            rb_sb = work.tile([128, R], F32, tag="rb")
            nc.scalar.activation(out=rb_sb, in_=rb_ps, func=AF.Copy)

            aggt_sb = big.tile([128, 2, R], F32R, tag="aggt")
            for i in range(2):
                nrm_sb = work.tile([128, R], F32, tag="nrm")
                nc.vector.tensor_mul(nrm_sb, agg_pss[i], rb_sb)
                nc.scalar.activation(out=aggt_sb[:, i, :], in_=nrm_sb, func=AF.Sigmoid)

            # ---- final fc: outT = sigmoid(fcw.T @ [xT; aggT] + b) ----
            outt_sb = big.tile([128, 2, R], F32, tag="outt")
            for j in range(2):
                fc_ps = psB.tile([128, R], F32, tag="scfc")
                for k in range(2):
                    nc.tensor.matmul(
                        out=fc_ps,
                        lhsT=fcw_sb[:, k, j * 128:(j + 1) * 128],
                        rhs=xt_sb[:, k, :],
                        start=(k == 0), stop=False,
                    )
                for k in range(2):
                    nc.tensor.matmul(
                        out=fc_ps,
                        lhsT=fcw_sb[:, 2 + k, j * 128:(j + 1) * 128],
                        rhs=aggt_sb[:, k, :],
                        start=False, stop=(k == 1),
                    )
                nc.scalar.activation(
                    out=outt_sb[:, j, :], in_=fc_ps, func=AF.Sigmoid,
                    bias=fcb_sb[:, j, :],
                )
                nc.sync.dma_start(out=out[:, j, :], in_=outt_sb[:, j, :])

    nc.finalize()
    return nc


def _chunk(a):
    """[K, N] with K = c*128  ->  [128, c, N] contiguous."""
    k, n = a.shape
    c = k // 128
    return np.ascontiguousarray(a.reshape(c, 128, n).transpose(1, 0, 2))


_NC_CACHE = None


def kernel(**inputs):
    global _NC_CACHE
    if _NC_CACHE is None:
        _NC_CACHE = build_kernel()
    nc = _NC_CACHE

    x = np.asarray(inputs["x"], np.float32)
    neibs_full = np.asarray(inputs["neibs"], np.float32)
    mask = np.asarray(inputs["mask"], np.float32)

    shared = {
        "neibst": _chunk(np.ascontiguousarray(neibs_full.T)),
        "neibs": _chunk(neibs_full),
        "axw1": _chunk(np.asarray(inputs["ax_w1"], np.float32)),
        "axb1": np.asarray(inputs["ax_b1"], np.float32).reshape(H, 1),
        "axw2": np.ascontiguousarray(np.asarray(inputs["ax_w2"], np.float32)),
        "axb2": np.asarray(inputs["ax_b2"], np.float32).reshape(H, 1),
        "anw1": _chunk(np.asarray(inputs["an_w1"], np.float32)),
        "anb1": np.asarray(inputs["an_b1"], np.float32).reshape(H, 1),
        "anw2": np.ascontiguousarray(np.asarray(inputs["an_w2"], np.float32)),
        "anb2": np.asarray(inputs["an_b2"], np.float32).reshape(H, 1),
        "fcw": _chunk(np.asarray(inputs["fcx_w"], np.float32)),
        "fcb": _chunk(np.asarray(inputs["fcx_b"], np.float32).reshape(2 * 128, 1)),
        "onescol": np.ones((128, 1), np.float32),
        "onesrow": np.ones((1, 128), np.float32),
    }

    in_maps = []
    for c in range(NCORES):
        rows = slice(c * R, (c + 1) * R)
        m = dict(shared)
        m["xt"] = _chunk(np.ascontiguousarray(x[rows].T))
        m["maskt"] = _chunk(np.ascontiguousarray(mask[rows].T))
        in_maps.append(m)

    global _last_in_maps
    _last_in_maps = in_maps
    res = run_bass_kernel_spmd(nc, in_maps, core_ids=list(range(NCORES)))

    out = np.empty((N, DO), np.float32)
    for c in range(NCORES):
        ot = res.results[c]["out"]          # [128, 2, R]
        outt = ot.transpose(1, 0, 2).reshape(DO, R)   # out_shard.T
        out[c * R:(c + 1) * R] = outt.T
    return out


# revision 32
# speedup vs baseline: 1345.7028x; 1345.7028x over previous
"""Trainium2 Bass kernel for nn_AttentionAggregator3 (GNN message passing).

Takes FULL unsharded inputs, shards target-node rows (N=2048) across 8
NeuronCores (256 rows each), replicates neibs + weights, runs an
all-transposed dataflow per core, gathers the full (2048, 256) output.

Sigmoids are computed as 0.5*tanh(z/2)+0.5 so the ACT engine needs only one
function-set load; the affine for agg's sigmoid is folded into host-prescaled
fc weights and bias. All small weights ship in two packed blob DMAs.

Note: the reference computes an edge-embedding MLP whose result is never
used (dead code), so edge_emb is accepted and ignored.
"""

import os
import sys

sys.path.insert(0, "/opt/trn_rl_repo")
# recover automatically if a previous run left the NeuronCores wedged
os.environ.setdefault("NEURON_RT_RESET_CORES", "1")

import ml_dtypes
import numpy as np

import concourse.bass as bass
import concourse.tile as tile
from concourse import bacc, mybir
from concourse.bass_utils import run_bass_kernel_spmd

N = 2048        # target nodes
NN = 2048       # neighbor pool
D = 256         # d_in
H = 32          # hidden
DO = 256        # d_out
NCORES = 8
R = N // NCORES  # rows per core = 256
WINDOWS = ((0, 512), (512, 512), (1024, 512), (1536, 512))

F32 = mybir.dt.float32
F32R = mybir.dt.float32r
BF16 = mybir.dt.bfloat16

# f32 blob column offsets
_C_AXW1, _C_AXW2, _C_AXB1, _C_AXB2, _C_FCB, _C_ONEC, _C_ONER = 0, 64, 96, 97, 98, 100, 101
_C_ANB1, _C_ANB2 = 229, 230
TOT_F = 231
# bf16 blob
_C_ANW1, _C_ANW2, _C_ONESM = 0, 64, 96
TOT_B = 224


def build_kernel():
    nc = bacc.Bacc("TRN2")

    # ---- DRAM I/O (per-core shapes; pre-tiled on host into [128, c, n]) ----
    xt = nc.dram_tensor("xt", [128, 2, R], F32R, kind="ExternalInput")            # x_shard.T
    neibst = nc.dram_tensor("neibst", [128, 2, NN], BF16, kind="ExternalInput")   # neibs.T
    neibs = nc.dram_tensor("neibs", [128, 16, D], BF16, kind="ExternalInput")      # neibs
    maskt = nc.dram_tensor("maskt", [128, 16, R], BF16, kind="ExternalInput")      # mask_shard.T
    pf = nc.dram_tensor("pf", [128, TOT_F], F32R, kind="ExternalInput")            # packed f32 params
    pb = nc.dram_tensor("pb", [128, TOT_B], BF16, kind="ExternalInput")            # packed bf16 params
    fcw = nc.dram_tensor("fcw", [128, 4, DO], F32R, kind="ExternalInput")          # [fcw_x; 0.5*fcw_agg]
    out = nc.dram_tensor("out", [128, 2, R], F32, kind="ExternalOutput")           # out_shard.T

    AF = mybir.ActivationFunctionType

    with tile.TileContext(nc) as tc:
        with (
            tc.tile_pool(name="big", bufs=1) as big,
            tc.tile_pool(name="small", bufs=1) as small,
            tc.tile_pool(name="work", bufs=3) as work,
            tc.tile_pool(name="psA", bufs=2, space="PSUM") as psA,
            tc.tile_pool(name="psB", bufs=3, space="PSUM") as psB,
            tc.tile_pool(name="psC", bufs=1, space="PSUM") as psC,
        ):
            # ---- SBUF tiles ----
            xt_sb = big.tile([128, 2, R], F32R, tag="xt")
            neibst_sb = big.tile([128, 2, NN], BF16, tag="neibst")
            neibs_sb = big.tile([128, 16, D], BF16, tag="neibs")
            maskt_sb = big.tile([128, 16, R], BF16, tag="maskt")
            fcw_sb = big.tile([128, 4, DO], F32R, tag="fcw")
            pf_sb = small.tile([128, TOT_F], F32R, tag="pf")
            pb_sb = small.tile([128, TOT_B], BF16, tag="pb")

            # views into the packed blobs
            axw1_v = pf_sb[:, _C_AXW1:_C_AXW1 + 64].rearrange("p (k h) -> p k h", k=2)
            axw2_v = pf_sb[:H, _C_AXW2:_C_AXW2 + H]
            axb1_v = pf_sb[:H, _C_AXB1:_C_AXB1 + 1].bitcast(F32)
            axb2_v = pf_sb[:H, _C_AXB2:_C_AXB2 + 1].bitcast(F32)
            fcb_v = [pf_sb[:, _C_FCB + j:_C_FCB + j + 1].bitcast(F32) for j in range(2)]
            ones_m = pb_sb[:, _C_ONESM:_C_ONESM + 128]
            anw1_v = pb_sb[:, _C_ANW1:_C_ANW1 + 64].rearrange("p (k h) -> p k h", k=2)
            anw2_v = pb_sb[:H, _C_ANW2:_C_ANW2 + H]
            anb1_v = pf_sb[:H, _C_ANB1:_C_ANB1 + 1].bitcast(F32)
            anb2_v = pf_sb[:H, _C_ANB2:_C_ANB2 + 1].bitcast(F32)

            # ---- DMAs: criticality-ordered, split, spread over queues ----
            nc.scalar.dma_start(out=pf_sb, in_=pf[:])
            nc.sync.dma_start(out=xt_sb, in_=xt[:])
            nc.sync.dma_start(out=pb_sb, in_=pb[:])
            for h in range(2):
                nc.sync.dma_start(
                    out=neibs_sb[:, h * 8:(h + 1) * 8, :],
                    in_=neibs[:, h * 8:(h + 1) * 8, :],
                )
            for (wo, wn) in WINDOWS:
                nc.gpsimd.dma_start(
                    out=neibst_sb[:, :, wo:wo + wn], in_=neibst[:, :, wo:wo + wn])
            for h in range(2):
                nc.scalar.dma_start(
                    out=maskt_sb[:, h * 8:(h + 1) * 8, :],
                    in_=maskt[:, h * 8:(h + 1) * 8, :],
                )
            nc.scalar.dma_start(out=fcw_sb, in_=fcw[:])

            # ---- x_attT = (tanh(xT.T @ w1 + b1) @ w2 + b2).T   [H, R] ----
            h1x_ps = psA.tile([H, 512], F32, tag="mlp", name="h1x_ps")[:, :R]
            for k in range(2):
                nc.tensor.matmul(
                    out=h1x_ps, lhsT=axw1_v[:, k, :], rhs=xt_sb[:, k, :],
                    start=(k == 0), stop=(k == 1),
                )
            h1x_sb = small.tile([H, R], F32R, tag="h1x")
            nc.scalar.activation(out=h1x_sb, in_=h1x_ps, func=AF.Tanh, bias=axb1_v)
            xatt_ps = psA.tile([H, 512], F32, tag="mlp", name="xatt_ps")[:, :R]
            nc.tensor.matmul(
                out=xatt_ps, lhsT=axw2_v, rhs=h1x_sb, start=True, stop=True,
            )
            xatt_sb = small.tile([H, R], BF16, tag="xatt")
            nc.vector.tensor_scalar_add(xatt_sb, xatt_ps, axb2_v)

            # ---- neib_attT  [H, NN] (layer1/activations in bf16) ----
            h1n_sb = big.tile([H, NN], BF16, tag="h1n")
            natt_sb = big.tile([H, NN], BF16, tag="natt")
            for (wo, wn) in WINDOWS:
                h1n_ps = psA.tile([H, 512], F32, tag="mlp", name="h1n_ps")[:, :wn]
                for k in range(2):
                    nc.tensor.matmul(
                        out=h1n_ps, lhsT=anw1_v[:, k, :],
                        rhs=neibst_sb[:, k, wo:wo + wn],
                        start=(k == 0), stop=(k == 1),
                    )
                nc.scalar.activation(
                    out=h1n_sb[:, wo:wo + wn], in_=h1n_ps,
                    func=AF.Tanh, bias=anb1_v,
                )
                natt_ps = psA.tile([H, 512], F32, tag="mlp", name="natt_ps")[:, :wn]
                nc.tensor.matmul(
                    out=natt_ps, lhsT=anw2_v,
                    rhs=h1n_sb[:, wo:wo + wn],
                    start=True, stop=True,
                )
                nc.vector.tensor_scalar_add(
                    natt_sb[:, wo:wo + wn], natt_ps, anb2_v)

            # ---- scores -> e = exp(scores*mask); sums + aggT accumulate ----
            e_all = big.tile([128, 16, R], BF16, tag="e_all")
            acc_ps = psC.tile([128, 6, R], F32, tag="acc")  # agg i in bank i, sums bank 2
            sum_ps = acc_ps[:, 4, :]

            def emit_scores(g):
                sc_ps = psB.tile([128, 2, R], F32, tag="scfc", name="sc_ps")
                for mm in range(2):
                    m = 2 * g + mm
                    nc.tensor.matmul(
                        out=sc_ps[:, mm, :],
                        lhsT=natt_sb[:, m * 128:(m + 1) * 128],
                        rhs=xatt_sb[:],
                        start=True, stop=True,
                    )
                st_sb = work.tile([128, 2, R], F32, tag="st", name="st_sb")
                nc.vector.tensor_mul(st_sb, sc_ps, maskt_sb[:, 2 * g:2 * g + 2, :])
                nc.scalar.activation(
                    out=e_all[:, 2 * g:2 * g + 2, :], in_=st_sb, func=AF.Exp,
                )

            def emit_acc(g):
                for mm in range(2):
                    m = 2 * g + mm
                    nc.tensor.matmul(
                        out=sum_ps, lhsT=ones_m, rhs=e_all[:, m, :],
                        start=(m == 0), stop=(m == 15),
                    )
                    for i in range(2):
                        nc.tensor.matmul(
                            out=acc_ps[:, 2 * i, :],
                            lhsT=neibs_sb[:, m, i * 128:(i + 1) * 128],
                            rhs=e_all[:, m, :],
                            start=(m == 0), stop=(m == 15),
                        )

            emit_scores(0)
            emit_scores(1)
            emit_scores(2)
            for g in range(3, 8):
                emit_scores(g)
                emit_acc(g - 3)
            emit_acc(5)
            emit_acc(6)
            emit_acc(7)

            # ---- normalize + "sigmoid" (tanh form; affine folded into fcw) ----
            rb_sb = work.tile([128, R], F32, tag="rb")
            nc.vector.reciprocal(out=rb_sb, in_=sum_ps)

            acc4d = acc_ps.rearrange("p (a c) r -> p a c r", a=3)
            nrm_sb = work.tile([128, 2, R], F32, tag="nrm")
            nc.vector.tensor_mul(
                nrm_sb, acc4d[:, 0:2, 0, :],
                rb_sb.unsqueeze(1).broadcast_to([128, 2, R]),
            )
            aggt_sb = big.tile([128, 2, R], F32R, tag="aggt")
            nc.scalar.activation(out=aggt_sb, in_=nrm_sb, func=AF.Tanh, scale=0.5)

            outt_sb = big.tile([128, 2, R], F32, tag="outt")
            for j in range(2):
                fc_ps = psB.tile([128, 2, R], F32, tag="scfc", name="fc_ps")[:, 0, :]
                for k in range(2):
                    nc.tensor.matmul(
                        out=fc_ps, lhsT=fcw_sb[:, k, j * 128:(j + 1) * 128],
                        rhs=xt_sb[:, k, :], start=(k == 0), stop=False,
                    )
                for k in range(2):
                    nc.tensor.matmul(
                        out=fc_ps, lhsT=fcw_sb[:, 2 + k, j * 128:(j + 1) * 128],
                        rhs=aggt_sb[:, k, :], start=False, stop=(k == 1),
                    )
                th_sb = work.tile([128, R], F32, tag="th", name="th_sb")
                nc.scalar.activation(
                    out=th_sb, in_=fc_ps, func=AF.Tanh, scale=0.5, bias=fcb_v[j],
                )
                nc.vector.tensor_scalar(
                    out=outt_sb[:, j, :], in0=th_sb, scalar1=0.5, scalar2=0.5,
                    op0=mybir.AluOpType.mult, op1=mybir.AluOpType.add,
                )
                nc.sync.dma_start(out=out[:, j, :], in_=outt_sb[:, j, :])

    nc.finalize()
    return nc


def _chunk(a):
    """[K, N] with K = c*128  ->  [128, c, N] contiguous."""
    k, n = a.shape
    c = k // 128
    return np.ascontiguousarray(a.reshape(c, 128, n).transpose(1, 0, 2))


def _bf16(a):
    return np.ascontiguousarray(a.astype(ml_dtypes.bfloat16))


_NC_CACHE = None
_last_in_maps = None


def make_in_maps(inputs):
    x = np.asarray(inputs["x"], np.float32)
    neibs_full = np.asarray(inputs["neibs"], np.float32)
    mask = np.asarray(inputs["mask"], np.float32)
    fcw_full = np.asarray(inputs["fcx_w"], np.float32)      # [512, 256]
    fcb_full = np.asarray(inputs["fcx_b"], np.float32)      # [256]

    # fold agg's sigmoid affine (0.5*th + 0.5) into the fc weights/bias, and
    # pre-halve the bias for the tanh-form output sigmoid.
    fcw_mod = fcw_full.copy()
    fcw_mod[D:] *= 0.5
    fcb_mod = fcb_full + 0.5 * fcw_full[D:].sum(axis=0)
    fcb_half = 0.5 * fcb_mod
    fcb_ch = _chunk(fcb_half.reshape(2 * 128, 1))            # [128, 2, 1]

    # packed f32 blob [128, TOT_F]
    pf = np.zeros((128, TOT_F), np.float32)
    pf[:, _C_AXW1:_C_AXW1 + 64] = _chunk(
        np.asarray(inputs["ax_w1"], np.float32)).reshape(128, 64)
    pf[:H, _C_AXW2:_C_AXW2 + H] = np.asarray(inputs["ax_w2"], np.float32)
    pf[:H, _C_AXB1] = np.asarray(inputs["ax_b1"], np.float32)
    pf[:H, _C_AXB2] = np.asarray(inputs["ax_b2"], np.float32)
    pf[:, _C_FCB:_C_FCB + 2] = fcb_ch[:, :, 0]
    pf[:H, _C_ANB1] = np.asarray(inputs["an_b1"], np.float32)
    pf[:H, _C_ANB2] = np.asarray(inputs["an_b2"], np.float32)

    # packed bf16 blob [128, TOT_B]
    pbf = np.zeros((128, TOT_B), np.float32)
    pbf[:, _C_ANW1:_C_ANW1 + 64] = _chunk(
        np.asarray(inputs["an_w1"], np.float32)).reshape(128, 64)
    pbf[:H, _C_ANW2:_C_ANW2 + H] = np.asarray(inputs["an_w2"], np.float32)
    pbf[:, _C_ONESM:_C_ONESM + 128] = 1.0

    nt = _chunk(np.ascontiguousarray(neibs_full.T))          # [128, 2, 2048]

    shared = {
        "neibst": _bf16(nt),
        "neibs": _bf16(_chunk(neibs_full)),
        "pf": pf,
        "pb": _bf16(pbf),
        "fcw": _chunk(fcw_mod),
    }

    in_maps = []
    for c in range(NCORES):
        rows = slice(c * R, (c + 1) * R)
        m = dict(shared)
        m["xt"] = _chunk(np.ascontiguousarray(x[rows].T))
        m["maskt"] = _bf16(_chunk(np.ascontiguousarray(mask[rows].T)))
        in_maps.append(m)
    return in_maps


def kernel(**inputs):
    global _NC_CACHE, _last_in_maps
    if _NC_CACHE is None:
        _NC_CACHE = build_kernel()
    nc = _NC_CACHE

    in_maps = make_in_maps(inputs)
    _last_in_maps = in_maps
    res = run_bass_kernel_spmd(nc, in_maps, core_ids=list(range(NCORES)))

    out = np.empty((N, DO), np.float32)
    for c in range(NCORES):
        ot = res.results[c]["out"]          # [128, 2, R]
        outt = ot.transpose(1, 0, 2).reshape(DO, R)   # out_shard.T
        out[c * R:(c + 1) * R] = outt.T
    return out
